# revision 2
# baseline (speedup 1.0000x reference)
"""Multi-head causal+padded attention on 8 Trainium2 NeuronCores.

Sharding: core c handles batch b = c//2 and head-group g = c%2 (8 of 16 heads).
Each core computes its q/k/v projections (512 output dims) and attention for
its 8 heads over the full 2048-seq, producing out^T [512, 2048]; the host
transposes/concats into the full [4, 2048, 1024] output.

Device algorithm (per core):
  xT [1024,2048] resident in SBUF; qT/kT = W^T-slices @ xT (f32r matmuls,
  output layout [outdim, seq]); v in natural [seq, outdim] layout, bias-added,
  pad-masked, stored bf16 augmented with a 65th column = pad mask.
  Scores are computed transposed (sT[k,q] = k_h^T q_h) per 128-k-block, exp'd
  on the scalar engine (scale=1/8 folded in), causal-masked only on diagonal
  blocks, then att^T @ [v|pad] accumulates in PSUM giving both out^T[d,q] and
  the softmax denominator (row 64) in one accumulation chain. Normalization
  multiplies by the broadcast reciprocal (gpsimd partition_broadcast).
"""
import os
import sys

sys.path.insert(0, "/opt/trn_rl_repo")

import numpy as np

S = 2048
E = 1024
D = 64
H = 16          # total heads
HPC = 8         # heads per core
OC = HPC * D    # 512 output dims per core
EB = E // 128   # 8 contraction blocks
NSB = S // 128  # 16 seq blocks
NCH = S // 512  # 4 q-chunks
B = 4
NCORES = 8

_cache = {}


def _build_nc():
    from concourse import bacc
    import concourse.tile as tile
    import concourse.mybir as mybir

    F32 = mybir.dt.float32
    F32R = mybir.dt.float32r
    ATT_DT = {"bf16": mybir.dt.bfloat16, "f32r": F32R}[
        os.environ.get("MHA_ATT_DT", "bf16")
    ]
    AF = mybir.ActivationFunctionType

    nc = bacc.Bacc("TRN2", target_bir_lowering=False, debug=False,
                   num_devices=NCORES)
    xT = nc.dram_tensor("xT", [E, S], F32R, kind="ExternalInput").ap()
    wqT = nc.dram_tensor("wqT", [E, OC], F32R, kind="ExternalInput").ap()
    wkT = nc.dram_tensor("wkT", [E, OC], F32R, kind="ExternalInput").ap()
    wvT = nc.dram_tensor("wvT", [E, OC], F32R, kind="ExternalInput").ap()
    bq = nc.dram_tensor("bq", [OC], F32, kind="ExternalInput").ap()
    bk = nc.dram_tensor("bk", [OC], F32, kind="ExternalInput").ap()
    bv = nc.dram_tensor("bv", [OC], F32, kind="ExternalInput").ap()
    pad = nc.dram_tensor("pad", [S], F32, kind="ExternalInput").ap()
    outT = nc.dram_tensor("outT", [OC, S], F32, kind="ExternalOutput").ap()

    with tile.TileContext(nc) as tc:
        with tc.tile_pool(name="const", bufs=1) as cpool, \
             tc.tile_pool(name="big", bufs=1) as bigpool:

            # ---------------- constants ----------------
            pad_sb = cpool.tile([128, NSB], F32, tag="pad_f")
            nc.sync.dma_start(pad_sb[:], pad.rearrange("(b p) -> p b", p=128))
            pad_row = cpool.tile([1, S], F32, tag="padr_f")
            nc.sync.dma_start(pad_row[:], pad.rearrange("(a s) -> a s", a=1))

            bq_sb = cpool.tile([128, 4], F32, tag="bq")
            nc.sync.dma_start(bq_sb[:], bq.rearrange("(b p) -> p b", p=128))
            bk_sb = cpool.tile([128, 4], F32, tag="bk")
            nc.sync.dma_start(bk_sb[:], bk.rearrange("(b p) -> p b", p=128))
            bv_row = cpool.tile([1, OC], F32, tag="bv_row")
            nc.sync.dma_start(bv_row[:], bv.rearrange("(a c) -> a c", a=1))
            bv_tile = cpool.tile([128, OC], F32, tag="bv_tile")
            nc.gpsimd.partition_broadcast(bv_tile[:], bv_row[:])

            # tri[k, q] = 1 where k <= q else 0 (local 128x128 diagonal block)
            tri = cpool.tile([128, 128], ATT_DT, tag="tri")
            nc.gpsimd.memset(tri[:], 1.0)
            nc.gpsimd.affine_select(
                out=tri[:], in_=tri[:], compare_op=mybir.AluOpType.is_ge,
                fill=0.0, base=0, pattern=[[1, 128]], channel_multiplier=-1)

            qT_sb = bigpool.tile([128, 4 * S], F32R, tag="qT")
            kT_sb = bigpool.tile([128, 4 * S], F32R, tag="kT")
            v_aug = bigpool.tile([128, NSB * HPC * 65], ATT_DT, tag="v_aug")
            v_r = v_aug[:].rearrange("p (b h c) -> p b h c", b=NSB, h=HPC)

            # ======== phase 1: projections ========
            with tc.tile_pool(name="xw", bufs=3) as xw, \
                 tc.tile_pool(name="xp", bufs=1) as xp, \
                 tc.tile_pool(name="psP", bufs=4, space="PSUM") as psP:

                x_sb = xp.tile([128, EB * S], F32R, tag="x_sb")
                for eb in range(EB):
                    nc.sync.dma_start(x_sb[:, eb * S:(eb + 1) * S],
                                      xT[eb * 128:(eb + 1) * 128, :])

                # init the 65th (pad) columns of v_aug once
                nc.gpsimd.memset(v_r[:, :, :, 64], 1.0)

                def load_w(wdram):
                    halves = []
                    for half in range(2):
                        w_sb = xw.tile([128, 4 * OC], F32R, tag="w",
                                       name=f"w_{half}")
                        for i in range(4):
                            eb = half * 4 + i
                            nc.sync.dma_start(
                                w_sb[:, i * OC:(i + 1) * OC],
                                wdram[eb * 128:(eb + 1) * 128, :])
                        halves.append(w_sb)
                    return halves

                # q/k projections (out layout [o, s])
                for wdram, bias_sb, dst in ((wqT, bq_sb, qT_sb),
                                            (wkT, bk_sb, kT_sb)):
                    wh = load_w(wdram)
                    for ob in range(4):
                        for scn in range(4):
                            ps = psP.tile([128, 512], F32, tag="ps_proj")
                            for eb in range(EB):
                                w_sb = wh[eb // 4]
                                i = eb % 4
                                nc.tensor.matmul(
                                    ps[:],
                                    w_sb[:, i * OC + ob * 128:
                                         i * OC + (ob + 1) * 128],
                                    x_sb[:, eb * S + scn * 512:
                                         eb * S + (scn + 1) * 512],
                                    start=(eb == 0), stop=(eb == EB - 1))
                            nc.vector.tensor_scalar_add(
                                dst[:, ob * S + scn * 512:
                                    ob * S + (scn + 1) * 512],
                                ps[:], bias_sb[:, ob:ob + 1])

                # v projection (natural [s, o] layout, bias+pad, bf16)
                wh = load_w(wvT)
                for sb in range(NSB):
                    ps = psP.tile([128, 512], F32, tag="ps_proj")
                    for eb in range(EB):
                        w_sb = wh[eb // 4]
                        i = eb % 4
                        nc.tensor.matmul(
                            ps[:],
                            x_sb[:, eb * S + sb * 128:eb * S + (sb + 1) * 128],
                            w_sb[:, i * OC:(i + 1) * OC],
                            start=(eb == 0), stop=(eb == EB - 1))
                    nc.vector.tensor_add(
                        v_r[:, sb, :, 0:64],
                        ps[:].rearrange("p (h c) -> p h c", h=HPC),
                        bv_tile[:].rearrange("p (h c) -> p h c", h=HPC))
                    nc.vector.tensor_scalar_mul(
                        v_aug[:, sb * HPC * 65:(sb + 1) * HPC * 65],
                        v_aug[:, sb * HPC * 65:(sb + 1) * HPC * 65],
                        pad_sb[:, sb:sb + 1])

            stage = os.environ.get("MHA_STAGE", "full")
            if stage == "proj":
                with tc.tile_pool(name="dbg", bufs=2) as dbg:
                    nc.sync.dma_start(outT[0:128, :], qT_sb[:, 0:S])
                    nc.sync.dma_start(outT[128:256, :], kT_sb[:, 0:S])
                    vdump = dbg.tile([128, 512], F32, tag="vd")
                    nc.vector.tensor_copy(vdump[:], v_aug[:, 0:512])
                    nc.sync.dma_start(outT[256:384, 0:512], vdump[:])
                nc.compile()
                return nc

            # ======== phase 2: attention ========
            with tc.tile_pool(name="attp", bufs=6) as attp, \
                 tc.tile_pool(name="work", bufs=4) as work, \
                 tc.tile_pool(name="outp", bufs=3) as outp, \
                 tc.tile_pool(name="psS", bufs=4, space="PSUM") as psS, \
                 tc.tile_pool(name="psAv", bufs=2, space="PSUM") as psAv:

                for scn in range(NCH):
                    q0 = scn * 512
                    nkb = 4 * scn + 4
                    for hp in range(4):
                        heads = (2 * hp, 2 * hp + 1)
                        if stage != "noav":
                            avs = [psAv.tile([65, 512], F32, tag=f"ps_av{i}",
                                             name=f"ps_av{i}")
                                   for i in range(2)]
                        for kb in range(nkb):
                            lstart = max(0, kb * 128 - q0)
                            w = 512 - lstart
                            for i, h in enumerate(heads):
                                ob = h // 2
                                po = (h % 2) * 64
                                ssb = psS.tile([128, 512], F32, tag="ps_s")
                                nc.tensor.matmul(
                                    ssb[:, 0:w],
                                    kT_sb[po:po + 64,
                                          ob * S + kb * 128:
                                          ob * S + (kb + 1) * 128],
                                    qT_sb[po:po + 64,
                                          ob * S + q0 + lstart:ob * S + q0 + 512],
                                    start=True, stop=True)
                                att_t = attp.tile([128, 512], ATT_DT, tag="att")
                                nc.scalar.activation(att_t[:, 0:w], ssb[:, 0:w],
                                                     AF.Exp, scale=0.125)
                                if kb >= 4 * scn:
                                    nc.vector.tensor_mul(att_t[:, 0:128],
                                                         att_t[:, 0:128],
                                                         tri[:])
                                if stage != "noav":
                                    nc.tensor.matmul(
                                        avs[i][:, lstart:512],
                                        v_r[:, kb, h, :],
                                        att_t[:, 0:w],
                                        start=(kb == 0), stop=(kb == nkb - 1))
                                elif kb == nkb - 1:
                                    o_sb = outp.tile([64, 512], F32, tag="osb",
                                                     name="o_sb")
                                    nc.vector.tensor_copy(o_sb[:],
                                                          att_t[0:64, :])
                                    nc.sync.dma_start(
                                        outT[h * 64:(h + 1) * 64, q0:q0 + 512],
                                        o_sb[:])
                        if stage == "noav":
                            continue
                        for i, h in enumerate(heads):
                            r0 = work.tile([1, 512], F32, tag="rt", name="r0")
                            nc.vector.tensor_scalar_add(r0[:], avs[i][64:65, :],
                                                        1e-30)
                            r1 = work.tile([1, 512], F32, tag="rt", name="r1")
                            nc.vector.reciprocal(r1[:], r0[:])
                            r2 = work.tile([1, 512], F32, tag="rt", name="r2")
                            nc.vector.tensor_mul(r2[:], r1[:],
                                                 pad_row[:, q0:q0 + 512])
                            o_sb = outp.tile([64, 512], F32, tag="osb",
                                             name="o_sb")
                            if stage == "nobc":
                                nc.vector.tensor_copy(o_sb[:], avs[i][0:64, :])
                            else:
                                bc = work.tile([64, 512], F32, tag="bc",
                                               name="bc")
                                nc.gpsimd.partition_broadcast(bc[:], r2[:])
                                nc.vector.tensor_mul(o_sb[:], avs[i][0:64, :],
                                                     bc[:])
                            nc.sync.dma_start(
                                outT[h * 64:(h + 1) * 64, q0:q0 + 512],
                                o_sb[:])
    nc.compile()
    return nc


def get_nc():
    key = (os.environ.get("MHA_ATT_DT", "bf16"),
           os.environ.get("MHA_STAGE", "full"))
    if key not in _cache:
        _cache[key] = _build_nc()
    return _cache[key]


def make_in_maps(input_x, pad_mask, Wq, bq, Wk, bk, Wv, bv):
    input_x = np.asarray(input_x, dtype=np.float32)
    pad_f = np.asarray(pad_mask).astype(np.float32)
    Wq = np.asarray(Wq, dtype=np.float32)
    Wk = np.asarray(Wk, dtype=np.float32)
    Wv = np.asarray(Wv, dtype=np.float32)
    bq = np.asarray(bq, dtype=np.float32)
    bk = np.asarray(bk, dtype=np.float32)
    bv = np.asarray(bv, dtype=np.float32)

    xTs = [np.ascontiguousarray(input_x[b].T) for b in range(B)]
    wslices = {}
    for g in range(2):
        sl = slice(g * OC, (g + 1) * OC)
        wslices[g] = (np.ascontiguousarray(Wq[sl].T),
                      np.ascontiguousarray(Wk[sl].T),
                      np.ascontiguousarray(Wv[sl].T),
                      np.ascontiguousarray(bq[sl]),
                      np.ascontiguousarray(bk[sl]),
                      np.ascontiguousarray(bv[sl]))
    in_maps = []
    for c in range(NCORES):
        b, g = c // 2, c % 2
        wq_t, wk_t, wv_t, bq_s, bk_s, bv_s = wslices[g]
        in_maps.append({
            "xT": xTs[b], "wqT": wq_t, "wkT": wk_t, "wvT": wv_t,
            "bq": bq_s, "bk": bk_s, "bv": bv_s,
            "pad": np.ascontiguousarray(pad_f[b]),
        })
    return in_maps


def assemble(results):
    out = np.empty((B, S, E), dtype=np.float32)
    for c in range(NCORES):
        b, g = c // 2, c % 2
        out[b, :, g * OC:(g + 1) * OC] = results[c]["outT"].T
    return out


_last_result = None


def kernel(input_x, pad_mask, Wq, bq, Wk, bk, Wv, bv):
    global _last_result
    from concourse.bass_utils import run_bass_kernel_spmd
    nc = get_nc()
    in_maps = make_in_maps(input_x, pad_mask, Wq, bq, Wk, bk, Wv, bv)
    res = run_bass_kernel_spmd(nc, in_maps, core_ids=list(range(NCORES)))
    _last_result = res
    if res.exec_time_ns is not None:
        print(f"HW exec time: {res.exec_time_ns} ns")
    return assemble(res.results)



# revision 5
# speedup vs baseline: 1.8447x; 1.8447x over previous
"""Multi-head causal+padded attention on 8 Trainium2 NeuronCores.

Sharding: core c handles batch b = c//2 and head-group g = c%2 (8 of 16 heads).
Each core computes its q/k/v projections (512 output dims) and attention for
its 8 heads over the full 2048-seq, producing out^T [512, 2048]; the host
transposes/concats into the full [4, 2048, 1024] output.

Device algorithm (per core), v2:
  All matmul operands bf16 (fp32 PSUM accumulation). xT resident in SBUF;
  qT/kT = W^T-slices @ xT ([outdim, seq] layout); v natural [seq, outdim],
  augmented with a 65th all-ones column whose att-weighted sum is the softmax
  denominator. Key-side pad masking is folded into the exp bias (-87 for
  padded keys). Scores for a head pair are computed as two row-tiled matmuls
  (partitions 0-63 / 64-127 of the contraction) into adjacent PSUM banks and
  exp'd by a single paired ACT instruction. AV matmuls run one k-block behind
  QK/exp (software pipeline). Projections for seq-chunk scn+1 are interleaved
  into the attention of chunk scn as micro-ops to keep the PE busy while the
  ACT engine drains exp work. Normalization: copy PSUM->SBUF, fast-approx
  reciprocal of the denominator row, pad(q) fold, gpsimd partition broadcast,
  one elementwise multiply.
"""
import os
import sys

sys.path.insert(0, "/opt/trn_rl_repo")

import numpy as np
import ml_dtypes

S = 2048
E = 1024
D = 64
H = 16          # total heads
HPC = 8         # heads per core
OC = HPC * D    # 512 output dims per core
EB = E // 128   # 8 contraction blocks
NSB = S // 128  # 16 seq blocks
NCH = S // 512  # 4 q-chunks
B = 4
NCORES = 8

_cache = {}


def _build_nc():
    from concourse import bacc
    import concourse.tile as tile
    import concourse.mybir as mybir

    F32 = mybir.dt.float32
    BF16 = mybir.dt.bfloat16
    AF = mybir.ActivationFunctionType
    ALU = mybir.AluOpType

    nc = bacc.Bacc("TRN2", target_bir_lowering=False, debug=False,
                   num_devices=NCORES)
    xT = nc.dram_tensor("xT", [E, S], BF16, kind="ExternalInput").ap()
    wqT = nc.dram_tensor("wqT", [E, OC], BF16, kind="ExternalInput").ap()
    wkT = nc.dram_tensor("wkT", [E, OC], BF16, kind="ExternalInput").ap()
    wvT = nc.dram_tensor("wvT", [E, OC], BF16, kind="ExternalInput").ap()
    bq = nc.dram_tensor("bq", [OC], F32, kind="ExternalInput").ap()
    bk = nc.dram_tensor("bk", [OC], F32, kind="ExternalInput").ap()
    bv = nc.dram_tensor("bv", [OC], F32, kind="ExternalInput").ap()
    pad = nc.dram_tensor("pad", [S], F32, kind="ExternalInput").ap()
    outT = nc.dram_tensor("outT", [OC, S], F32, kind="ExternalOutput").ap()

    with tile.TileContext(nc) as tc:
        with tc.tile_pool(name="const", bufs=1) as cpool, \
             tc.tile_pool(name="big", bufs=1) as bigpool:

            # ---------------- constants ----------------
            pad_sb = cpool.tile([128, NSB], F32, tag="pad_f")
            nc.sync.dma_start(pad_sb[:], pad.rearrange("(b p) -> p b", p=128))
            pad_row = cpool.tile([1, S], F32, tag="padr_f")
            nc.sync.dma_start(pad_row[:], pad.rearrange("(a s) -> a s", a=1))
            # exp bias: 0 where pad=1, -87 where pad=0 (folds key padding)
            padlog = cpool.tile([128, NSB], F32, tag="padlog")
            nc.vector.tensor_scalar(padlog[:], pad_sb[:], 87.0, -87.0,
                                    ALU.mult, ALU.add)

            bq_sb = cpool.tile([128, 4], F32, tag="bq")
            nc.sync.dma_start(bq_sb[:], bq.rearrange("(b p) -> p b", p=128))
            bk_sb = cpool.tile([128, 4], F32, tag="bk")
            nc.sync.dma_start(bk_sb[:], bk.rearrange("(b p) -> p b", p=128))
            bv_row = cpool.tile([1, OC], F32, tag="bv_row")
            nc.sync.dma_start(bv_row[:], bv.rearrange("(a c) -> a c", a=1))
            bv_tile = cpool.tile([128, OC], F32, tag="bv_tile")
            nc.gpsimd.partition_broadcast(bv_tile[:], bv_row[:])

            # tri[k, q] = 1 where k <= q else 0 (local 128x128 diagonal block)
            tri = cpool.tile([128, 128], BF16, tag="tri")
            nc.gpsimd.memset(tri[:], 1.0)
            nc.gpsimd.affine_select(
                out=tri[:], in_=tri[:], compare_op=ALU.is_ge,
                fill=0.0, base=0, pattern=[[1, 128]], channel_multiplier=-1)

            # ---------------- big SBUF residents ----------------
            x_sb = bigpool.tile([128, EB * S], BF16, tag="x_sb")
            wq_sb = bigpool.tile([128, EB * OC], BF16, tag="wq_sb")
            wk_sb = bigpool.tile([128, EB * OC], BF16, tag="wk_sb")
            wv_sb = bigpool.tile([128, EB * OC], BF16, tag="wv_sb")
            qT_sb = bigpool.tile([128, 4 * S], BF16, tag="qT")
            kT_sb = bigpool.tile([128, 4 * S], BF16, tag="kT")
            v_aug = bigpool.tile([128, NSB * HPC * 65], BF16, tag="v_aug")
            v_r = v_aug[:].rearrange("p (b h c) -> p b h c", b=NSB, h=HPC)

            # wq first so the first q-projection group is paced only by the
            # x-block arrivals, then x, then wk/wv (needed later).
            for eb in range(EB):
                nc.sync.dma_start(wq_sb[:, eb * OC:(eb + 1) * OC],
                                  wqT[eb * 128:(eb + 1) * 128, :])
            for eb in range(EB):
                nc.sync.dma_start(x_sb[:, eb * S:(eb + 1) * S],
                                  xT[eb * 128:(eb + 1) * 128, :])
            for eb in range(EB):
                nc.sync.dma_start(wk_sb[:, eb * OC:(eb + 1) * OC],
                                  wkT[eb * 128:(eb + 1) * 128, :])
            for eb in range(EB):
                nc.sync.dma_start(wv_sb[:, eb * OC:(eb + 1) * OC],
                                  wvT[eb * 128:(eb + 1) * 128, :])

            # denominator column (65th) is constant 1; padding handled in exp
            nc.gpsimd.memset(v_r[:, :, :, 64], 1.0)

            with tc.tile_pool(name="psP", bufs=2, space="PSUM") as psP, \
                 tc.tile_pool(name="psS", bufs=2, space="PSUM") as psS, \
                 tc.tile_pool(name="psAv", bufs=1, space="PSUM") as psAv, \
                 tc.tile_pool(name="attp", bufs=4) as attp, \
                 tc.tile_pool(name="outp", bufs=3) as outp, \
                 tc.tile_pool(name="wkp", bufs=6) as wkp, \
                 tc.tile_pool(name="bcp", bufs=3) as bcp:

                def proj_gen(scn):
                    """Emit projection matmuls for q/k chunk scn and v blocks
                    4*scn..4*scn+4, yielding after each instruction so the
                    caller can interleave."""
                    q0 = scn * 512
                    for ob in range(4):
                        for wsb, bias_sb, dst in ((wq_sb, bq_sb, qT_sb),
                                                  (wk_sb, bk_sb, kT_sb)):
                            ps = psP.tile([128, 512], F32, tag="ps_proj")
                            for eb in range(EB):
                                nc.tensor.matmul(
                                    ps[:],
                                    wsb[:, eb * OC + ob * 128:
                                        eb * OC + (ob + 1) * 128],
                                    x_sb[:, eb * S + q0:eb * S + q0 + 512],
                                    start=(eb == 0), stop=(eb == EB - 1))
                                yield
                            nc.vector.tensor_scalar_add(
                                dst[:, ob * S + q0:ob * S + q0 + 512],
                                ps[:], bias_sb[:, ob:ob + 1])
                            yield
                    for sb in range(4 * scn, 4 * scn + 4):
                        ps = psP.tile([128, 512], F32, tag="ps_proj")
                        for eb in range(EB):
                            nc.tensor.matmul(
                                ps[:],
                                x_sb[:, eb * S + sb * 128:
                                     eb * S + (sb + 1) * 128],
                                wv_sb[:, eb * OC:(eb + 1) * OC],
                                start=(eb == 0), stop=(eb == EB - 1))
                            yield
                        nc.vector.tensor_add(
                            v_r[:, sb, :, 0:64],
                            ps[:].rearrange("p (h c) -> p h c", h=HPC),
                            bv_tile[:].rearrange("p (h c) -> p h c", h=HPC))
                        yield

                # projection unit size in yields: 16 qk-groups*9 + 4 v-groups*9
                PROJ_OPS = 16 * 9 + 4 * 9  # 180

                # ---- chunk 0 projections up front ----
                for _ in proj_gen(0):
                    pass

                # ---- attention scn with proj(scn+1) interleaved ----
                for scn in range(NCH):
                    q0 = scn * 512
                    nkb = 4 * scn + 4
                    gen = proj_gen(scn + 1) if scn + 1 < NCH else iter(())
                    ops_left = PROJ_OPS if scn + 1 < NCH else 0
                    slots_left = 4 * nkb

                    for hp in range(4):
                        heads = (2 * hp, 2 * hp + 1)
                        avs2 = psAv.tile([65, 1024], F32, tag="ps_av")
                        prev = None
                        for kb in range(nkb):
                            lstart = max(0, kb * 128 - q0)
                            w = 512 - lstart
                            ps2 = psS.tile([128, 1024], F32, tag="ps_s")
                            for i, h in enumerate(heads):
                                ob = h // 2
                                po = (h % 2) * 64
                                nc.tensor.matmul(
                                    ps2[:, i * 512:i * 512 + w],
                                    kT_sb[po:po + 64,
                                          ob * S + kb * 128:
                                          ob * S + (kb + 1) * 128],
                                    qT_sb[po:po + 64,
                                          ob * S + q0 + lstart:
                                          ob * S + q0 + 512],
                                    start=True, stop=True,
                                    tile_position=(po, 0))
                            att2 = attp.tile([128, 1024], BF16, tag="att")
                            if w == 512:
                                nc.scalar.activation(
                                    att2[:], ps2[:], AF.Exp, scale=0.125,
                                    bias=padlog[:, kb:kb + 1])
                            else:
                                pv = ps2[:].rearrange(
                                    "p (t c) -> p t c", t=2)[:, :, 0:w]
                                av = att2[:].rearrange(
                                    "p (t c) -> p t c", t=2)[:, :, 0:w]
                                nc.scalar.activation(
                                    av, pv, AF.Exp, scale=0.125,
                                    bias=padlog[:, kb:kb + 1])
                            if kb >= 4 * scn:
                                for i in range(2):
                                    nc.vector.tensor_mul(
                                        att2[:, i * 512:i * 512 + 128],
                                        att2[:, i * 512:i * 512 + 128],
                                        tri[:])
                            # interleave some projection work for scn+1
                            if ops_left > 0:
                                n = -(-ops_left // slots_left)
                                for _ in range(n):
                                    next(gen, None)
                                ops_left -= n
                            slots_left -= 1
                            if prev is not None:
                                p_att, p_lstart, p_w, p_kb = prev
                                for i, h in enumerate(heads):
                                    nc.tensor.matmul(
                                        avs2[:, i * 512 + p_lstart:
                                             i * 512 + 512],
                                        v_r[:, p_kb, h, :],
                                        p_att[:, i * 512:i * 512 + p_w],
                                        start=(p_kb == 0),
                                        stop=(p_kb == nkb - 1))
                            prev = (att2, lstart, w, kb)
                        p_att, p_lstart, p_w, p_kb = prev
                        for i, h in enumerate(heads):
                            nc.tensor.matmul(
                                avs2[:, i * 512 + p_lstart:i * 512 + 512],
                                v_r[:, p_kb, h, :],
                                p_att[:, i * 512:i * 512 + p_w],
                                start=(p_kb == 0), stop=(p_kb == nkb - 1))

                        # ---- normalize + store ----
                        av_sb2 = outp.tile([65, 1024], F32, tag="av_sb")
                        nc.vector.tensor_copy(av_sb2[:], avs2[:])
                        for i, h in enumerate(heads):
                            av_sb = av_sb2[:, i * 512:(i + 1) * 512]
                            rden = wkp.tile([1, 512], F32, tag="rt",
                                            name="rden")
                            nc.vector.tensor_scalar_add(
                                rden[:], av_sb2[64:65, i * 512:(i + 1) * 512],
                                1e-30)
                            rrec = wkp.tile([1, 512], F32, tag="rt",
                                            name="rrec")
                            nc.vector.reciprocal_approx_fast(rrec[:], rden[:])
                            rpad = wkp.tile([1, 512], F32, tag="rt",
                                            name="rpad")
                            nc.vector.tensor_mul(rpad[:], rrec[:],
                                                 pad_row[:, q0:q0 + 512])
                            bc = bcp.tile([64, 512], F32, tag="bc")
                            nc.gpsimd.partition_broadcast(bc[:], rpad[:])
                            nc.vector.tensor_mul(
                                av_sb2[0:64, i * 512:(i + 1) * 512],
                                av_sb2[0:64, i * 512:(i + 1) * 512], bc[:])
                            nc.sync.dma_start(
                                outT[h * 64:(h + 1) * 64, q0:q0 + 512],
                                av_sb2[0:64, i * 512:(i + 1) * 512])

                    # drain any leftover projection micro-ops for scn+1
                    for _ in gen:
                        pass
    nc.compile()
    return nc


def get_nc():
    if "nc" not in _cache:
        _cache["nc"] = _build_nc()
    return _cache["nc"]


def make_in_maps(input_x, pad_mask, Wq, bq, Wk, bk, Wv, bv):
    bf16 = ml_dtypes.bfloat16
    input_x = np.asarray(input_x, dtype=np.float32)
    pad_f = np.asarray(pad_mask).astype(np.float32)
    Wq = np.asarray(Wq, dtype=np.float32)
    Wk = np.asarray(Wk, dtype=np.float32)
    Wv = np.asarray(Wv, dtype=np.float32)
    bq = np.asarray(bq, dtype=np.float32)
    bk = np.asarray(bk, dtype=np.float32)
    bv = np.asarray(bv, dtype=np.float32)

    xTs = [np.ascontiguousarray(input_x[b].T).astype(bf16) for b in range(B)]
    wslices = {}
    for g in range(2):
        sl = slice(g * OC, (g + 1) * OC)
        wslices[g] = (np.ascontiguousarray(Wq[sl].T).astype(bf16),
                      np.ascontiguousarray(Wk[sl].T).astype(bf16),
                      np.ascontiguousarray(Wv[sl].T).astype(bf16),
                      np.ascontiguousarray(bq[sl]),
                      np.ascontiguousarray(bk[sl]),
                      np.ascontiguousarray(bv[sl]))
    in_maps = []
    for c in range(NCORES):
        b, g = c // 2, c % 2
        wq_t, wk_t, wv_t, bq_s, bk_s, bv_s = wslices[g]
        in_maps.append({
            "xT": xTs[b], "wqT": wq_t, "wkT": wk_t, "wvT": wv_t,
            "bq": bq_s, "bk": bk_s, "bv": bv_s,
            "pad": np.ascontiguousarray(pad_f[b]),
        })
    return in_maps


def assemble(results):
    out = np.empty((B, S, E), dtype=np.float32)
    for c in range(NCORES):
        b, g = c // 2, c % 2
        out[b, :, g * OC:(g + 1) * OC] = results[c]["outT"].T
    return out


_last_result = None


def kernel(input_x, pad_mask, Wq, bq, Wk, bk, Wv, bv):
    global _last_result
    from concourse.bass_utils import run_bass_kernel_spmd
    nc = get_nc()
    in_maps = make_in_maps(input_x, pad_mask, Wq, bq, Wk, bk, Wv, bv)
    res = run_bass_kernel_spmd(nc, in_maps, core_ids=list(range(NCORES)))
    _last_result = res
    if res.exec_time_ns is not None:
        print(f"HW exec time: {res.exec_time_ns} ns")
    return assemble(res.results)
